# revision 27
# baseline (speedup 1.0000x reference)
"""Trainium2 Bass kernel for nn_DynamicGroup_65377992180033 (moe_routing).

Computes, for B=64, H=1024, I=512:
    tau  = max(temperature, 1e-3)
    ic   = x_t @ W_ih.T + b_ih                      # (B, H)
    y    = softmax(W_hh/tau + gumbel_noise, axis=2) # (B, H, H)
    h    = tanh(ic + einsum('boh,bh->bo', y, h_prev))

Sharding over 8 NeuronCores: o-axis (rows of W_hh) split in 4 blocks of 256,
batch split in 2 halves of 32 -> core c handles (o-quarter c//2, b-half c%2).

16-bit dataflow (error budget: rel gate is 2e-2, measured ~1e-3):
  - gumbel is uploaded as fp16 (halves the 32 MB/core HBM stream, the
    dominant roofline term; |g| <= 18.4 so fp16 quantization <= 9e-3 abs).
  - DVE adds W_hh/tau (fp16) to g in fp16 at 2x DVE rate.
  - TensorE transposes the fp16 logits (1 cyc/row vs 2 for fp32) into fp16
    PSUM; one sample's 16 transposes fill a 2-bank PSUM tile.
  - ScalarE computes exp PSUM->SBUF as ONE 2048-elem activation per sample
    (32 instructions total; per-instruction overhead ~0.2us was the old
    Scalar bottleneck), output bf16 (exp <= 1.2e8 overflows fp16).
  - TensorE contracts E_T (bf16, 1 cyc/row) with per-sample stationaries
    [h_prev_b | ones] in bf16, 4 samples in parallel PE column groups.
  - Tail (fp32): transpose num/den back, divide, add ic, tanh, write out.
    PSUM->SBUF drains ride on DVE, not ScalarE.
"""
import numpy as np
import ml_dtypes
import bass_rust
import concourse.bass as bass
import concourse.tile as tile
from concourse import mybir
from concourse.bass_utils import run_bass_kernel_spmd

F32 = mybir.dt.float32
F16 = mybir.dt.float16
BF16 = mybir.dt.bfloat16
AF = mybir.ActivationFunctionType

B, H, I = 64, 1024, 512
NCORES = 8
OBLK = 2      # o-blocks of 128 per core -> 256 o-rows
BLOC = 32     # samples per core
KCH = 8       # h chunks of 128
MIN_TAU = 1e-3

# Results of the last run_bass_kernel_spmd call (for test harnesses to read
# exec_time_ns when run with BASS_TRACE=1).
LAST_RESULTS = None


def _split_multiwait_instructions(nc):
    """The walrus build here encodes at most one sync-wait per instruction.
    Move extra waits onto single-wait NoOps inserted just before, same
    engine, preserving program order (semantically identical)."""
    for f in nc.m.functions:
        for blk in f.blocks:
            out = []
            changed = False
            for inst in blk.instructions:
                si = inst.sync_info
                if si is not None and si.on_wait and len(si.on_wait) > 1:
                    waits = list(si.on_wait)
                    updates = list(si.on_update or [])
                    for j, w in enumerate(waits[:-1]):
                        nop = mybir.InstNoOp(name=f"{inst.name}-ws{j}", ins=[], outs=[])
                        nop.engine = inst.engine
                        nop.sync_info = bass_rust.SyncInfo(on_wait=[w], on_update=[])
                        out.append(nop)
                    inst.sync_info = bass_rust.SyncInfo(
                        on_wait=[waits[-1]], on_update=updates
                    )
                    changed = True
                out.append(inst)
            if changed:
                blk.instructions = out
    return nc


def _build(repeat=1):
    nc = bass.Bass()
    g_in = nc.dram_tensor("g_sl", [BLOC, OBLK * 128, H], F16, kind="ExternalInput")
    w_in = nc.dram_tensor("wT_sl", [H, OBLK * 128], F16, kind="ExternalInput")
    st_in = nc.dram_tensor("st_sl", [KCH, 128, 2 * BLOC], BF16, kind="ExternalInput")
    xt_in = nc.dram_tensor("xT_sl", [I, BLOC], F16, kind="ExternalInput")
    wih_in = nc.dram_tensor("wihT_sl", [I, OBLK * 128], F16, kind="ExternalInput")
    b_in = nc.dram_tensor("b_sl", [128, OBLK], F32, kind="ExternalInput")
    id32_in = nc.dram_tensor("ident32", [128, 128], F32, kind="ExternalInput")
    temp_in = nc.dram_tensor("temp", [128, 1], F32, kind="ExternalInput")
    h_out = nc.dram_tensor("h_sl", [BLOC, OBLK * 128], F32, kind="ExternalOutput")

    with tile.TileContext(nc) as tc:
        with (
            tc.tile_pool(name="cons", bufs=1) as cons,
            tc.tile_pool(name="lwork", bufs=3) as lwork,
            tc.tile_pool(name="ework", bufs=10) as ework,
            tc.tile_pool(name="tailsb", bufs=1) as tailsb,
            tc.tile_pool(name="acc_ps", bufs=4, space="PSUM") as acc_ps,
        ):
            # ---------------- setup ----------------
            # All stationaries ride the Scalar-engine HWDGE queue so the sync
            # queue carries nothing but the gumbel stream (its first dispatch
            # gates the whole pipeline).
            temp128 = cons.tile([128, 1], F32)
            nc.scalar.dma_start(temp128[:], temp_in[:])
            # W_hh^T slice, h on partitions (h = 128*c + p), matching the
            # XBAR DMA-transpose layout of the gumbel stream
            wt_sb = cons.tile([128, KCH, OBLK * 128], F16)
            nc.scalar.dma_start(
                wt_sb[:], w_in.ap().rearrange("(c p) o -> p c o", p=128)
            )
            # preload the activation table (Exp) early so the 1.3us
            # ACT_TABLE_LOAD doesn't sit in front of the first real exp
            dummy = cons.tile([128, 1], F32)
            nc.scalar.activation(dummy[:], temp128[:], AF.Exp)

            tau128 = cons.tile([128, 1], F32)
            nc.vector.tensor_scalar_max(tau128[:], temp128[:], MIN_TAU)
            rtau128 = cons.tile([128, 1], F32)
            nc.vector.reciprocal(rtau128[:], tau128[:])

            # Wtau^T = W_hh^T[o_blk] / tau, in fp16 for the 2x DVE add
            wtau_sb = cons.tile([128, KCH, OBLK * 128], F16)
            nc.vector.tensor_scalar_mul(wtau_sb[:], wt_sb[:], rtau128[:])

            ident32 = cons.tile([128, 128], F32)
            nc.scalar.dma_start(ident32[:], id32_in[:])
            st_sb = cons.tile([128, KCH, 2 * BLOC], BF16)
            nc.scalar.dma_start(st_sb[:], st_in.ap().rearrange("k p m -> p k m"))
            xt_sb = cons.tile([128, 4, BLOC], F16)
            nc.scalar.dma_start(
                xt_sb[:], xt_in.ap().rearrange("(k p) b -> p k b", p=128)
            )
            wih_sb = cons.tile([128, 4, OBLK * 128], F16)
            nc.scalar.dma_start(
                wih_sb[:], wih_in.ap().rearrange("(k p) o -> p k o", p=128)
            )
            bias_sb = cons.tile([128, OBLK], F32)
            nc.scalar.dma_start(bias_sb[:], b_in[:])

            # ic_T[i] = W_ih[o_blk_i] @ x^T + b  -> (128 o, BLOC b) per block
            # (runs in the PE preamble window before the first logits arrive)
            ic_sb = cons.tile([128, OBLK, BLOC], F32)
            for i in range(OBLK):
                ic_ps = acc_ps.tile([128, OBLK * 128], F32, tag="acc")
                for k in range(4):
                    nc.tensor.matmul(
                        ic_ps[:, :BLOC],
                        wih_sb[:, k, 128 * i : 128 * (i + 1)],
                        xt_sb[:, k, :],
                        start=(k == 0),
                        stop=(k == 3),
                    )
                nc.scalar.activation(
                    ic_sb[:, i, :], ic_ps[:, :BLOC], AF.Identity,
                    bias=bias_sb[:, i : i + 1],
                )

            ndg_all = tailsb.tile([128, BLOC // 4, OBLK * 128], F32)

            def _one_pass():
                contrib = tailsb.tile([128, OBLK, BLOC], F32)

                # ---- main loop: groups of 4 samples (PE column-groups),
                # contraction software-pipelined ONE GROUP BEHIND the
                # transpose+exp front so the PE never stalls on ScalarE ----
                def _contract(grp, ets):
                    acc = acc_ps.tile([128, OBLK * 128], F32, tag="acc")
                    for k in range(KCH):
                        for s in range(4):
                            b = 4 * grp + s
                            nc.tensor.matmul(
                                acc[32 * s : 32 * s + 2, :],
                                st_sb[:, k, 2 * b : 2 * b + 2],
                                ets[s][:, k, :],
                                start=(k == 0),
                                stop=(k == KCH - 1),
                                tile_position=(0, 32 * s),
                            )
                    nc.vector.tensor_copy(ndg_all[:, grp, :], acc[:])
                    # per-group tail, overlapped with the next group's work:
                    # transpose num/den back to o-partitions and divide
                    ndT_ps = acc_ps.tile([128, OBLK * 128], F32, tag="acc")
                    for i in range(OBLK):
                        nc.tensor.transpose(
                            ndT_ps[:, 128 * i : 128 * (i + 1)],
                            ndg_all[:, grp, 128 * i : 128 * (i + 1)],
                            ident32[:],
                        )
                    ndT = tailsb.tile([128, OBLK * 128], F32, bufs=2, tag="ndT")
                    nc.vector.tensor_copy(ndT[:], ndT_ps[:])
                    for i in range(OBLK):
                        rec = tailsb.tile([128, 4], F32, bufs=2, tag="rec")
                        nc.vector.reciprocal(
                            rec[:], ndT[:, 128 * i + 1 : 128 * (i + 1) : 32]
                        )
                        nc.vector.tensor_mul(
                            contrib[:, i, 4 * grp : 4 * grp + 4],
                            ndT[:, 128 * i : 128 * (i + 1) : 32],
                            rec[:],
                        )

                pending = None  # (grp, ets) whose contraction is deferred
                for grp in range(BLOC // 4):
                    ets = []
                    for s in range(4):
                        b = 4 * grp + s
                        # XBAR DMA-transpose: lands [h_part, o_free] directly
                        gt = lwork.tile([128, KCH, OBLK * 128], F16, bufs=8, tag="gt")
                        nc.sync.dma_start_transpose(gt[:], g_in.ap()[b])
                        lt = lwork.tile([128, KCH, OBLK * 128], F16, tag="lt")
                        nc.vector.tensor_add(lt[:], gt[:], wtau_sb[:])
                        # one 2048-elem exp per sample, SBUF fp16 -> SBUF bf16
                        eth = ework.tile([128, KCH, OBLK * 128], BF16, tag="eth")
                        nc.scalar.activation(eth[:], lt[:], AF.Exp)
                        ets.append(eth)

                    if pending is not None:
                        _contract(*pending)
                    pending = (grp, ets)
                _contract(*pending)

                # ---- final tail: tanh + output transpose ----
                hout = tailsb.tile([BLOC, OBLK, 128], F32)
                for i in range(OBLK):
                    hpre = tailsb.tile([128, BLOC], F32)
                    nc.vector.tensor_add(hpre[:], contrib[:, i, :], ic_sb[:, i, :])
                    ht = tailsb.tile([128, BLOC], F32)
                    nc.scalar.activation(ht[:], hpre[:], AF.Tanh)
                    hT_ps = acc_ps.tile([128, OBLK * 128], F32, tag="acc")
                    nc.tensor.transpose(
                        hT_ps[:BLOC, :128], ht[:], ident32[:]
                    )
                    nc.vector.tensor_copy(hout[:, i, :], hT_ps[:BLOC, :128])
                    nc.sync.dma_start(
                        h_out.ap()[:, 128 * i : 128 * (i + 1)], hout[:, i, :]
                    )

            for _rep in range(repeat):
                _one_pass()

    _split_multiwait_instructions(nc)
    return nc


def kernel(x_t, h_prev, W_ih, b_ih, W_hh, temperature, gumbel_noise):
    global LAST_RESULTS
    x_t = np.asarray(x_t, dtype=np.float32)
    h_prev = np.asarray(h_prev, dtype=np.float32)
    W_ih = np.asarray(W_ih, dtype=np.float32)
    b_ih = np.asarray(b_ih, dtype=np.float32)
    W_hh = np.asarray(W_hh, dtype=np.float32)
    temperature = np.asarray(temperature, dtype=np.float32)
    gumbel_noise = np.asarray(gumbel_noise, dtype=np.float32)

    nc = _build()

    ident32 = np.eye(128, dtype=np.float32)
    temp_arr = np.full((128, 1), float(temperature), dtype=np.float32)
    g16 = gumbel_noise.astype(np.float16)

    in_maps = []
    for c in range(NCORES):
        q, hb = divmod(c, 2)
        o0 = OBLK * 128 * q
        b0 = BLOC * hb
        g_sl = np.ascontiguousarray(g16[b0 : b0 + BLOC, o0 : o0 + OBLK * 128, :])
        wT_sl = np.ascontiguousarray(W_hh[o0 : o0 + OBLK * 128, :].T).astype(np.float16)
        st_sl = np.ones((KCH, 128, 2 * BLOC), np.float32)
        st_sl[:, :, 0::2] = np.ascontiguousarray(h_prev[b0 : b0 + BLOC].T).reshape(
            KCH, 128, BLOC
        )
        xT_sl = np.ascontiguousarray(x_t[b0 : b0 + BLOC].T).astype(np.float16)
        wihT_sl = np.ascontiguousarray(W_ih[o0 : o0 + OBLK * 128].T).astype(np.float16)
        b_sl = np.ascontiguousarray(b_ih[o0 : o0 + OBLK * 128].reshape(OBLK, 128).T)
        in_maps.append(
            {
                "g_sl": g_sl,
                "wT_sl": wT_sl,
                "st_sl": st_sl.astype(ml_dtypes.bfloat16),
                "xT_sl": xT_sl,
                "wihT_sl": wihT_sl,
                "b_sl": b_sl,
                "ident32": ident32,
                "temp": temp_arr,
            }
        )

    res = run_bass_kernel_spmd(nc, in_maps, list(range(NCORES)))
    LAST_RESULTS = res

    h = np.empty((B, H), np.float32)
    for c in range(NCORES):
        q, hb = divmod(c, 2)
        o0 = OBLK * 128 * q
        b0 = BLOC * hb
        h[b0 : b0 + BLOC, o0 : o0 + OBLK * 128] = res.results[c]["h_sl"]
    return h


# revision 39
# speedup vs baseline: 1.3640x; 1.3640x over previous
"""Trainium2 Bass kernel for nn_DynamicGroup_65377992180033 (moe_routing).

Computes, for B=64, H=1024, I=512:
    tau  = max(temperature, 1e-3)
    ic   = x_t @ W_ih.T + b_ih                      # (B, H)
    y    = softmax(W_hh/tau + gumbel_noise, axis=2) # (B, H, H)
    h    = tanh(ic + einsum('boh,bh->bo', y, h_prev))

Sharding over 8 NeuronCores: o-axis (rows of W_hh) split in 4 blocks of 256,
batch split in 2 halves of 32 -> core c handles (o-quarter c//2, b-half c%2).

16-bit dataflow (error budget: rel gate is 2e-2, measured ~1e-3):
  - gumbel is uploaded as fp16 (halves the 32 MB/core HBM stream, the
    dominant roofline term; |g| <= 18.4 so fp16 quantization <= 9e-3 abs).
  - DVE adds W_hh/tau (fp16) to g in fp16 at 2x DVE rate.
  - TensorE transposes the fp16 logits (1 cyc/row vs 2 for fp32) into fp16
    PSUM; one sample's 16 transposes fill a 2-bank PSUM tile.
  - ScalarE computes exp PSUM->SBUF as ONE 2048-elem activation per sample
    (32 instructions total; per-instruction overhead ~0.2us was the old
    Scalar bottleneck), output bf16 (exp <= 1.2e8 overflows fp16).
  - TensorE contracts E_T (bf16, 1 cyc/row) with per-sample stationaries
    [h_prev_b | ones] in bf16, 4 samples in parallel PE column groups.
  - Tail (fp32): transpose num/den back, divide, add ic, tanh, write out.
    PSUM->SBUF drains ride on DVE, not ScalarE.
"""
import numpy as np
import ml_dtypes
import bass_rust
import concourse.bass as bass
import concourse.tile as tile
from concourse import mybir
from concourse.bass_utils import run_bass_kernel_spmd

F32 = mybir.dt.float32
F16 = mybir.dt.float16
BF16 = mybir.dt.bfloat16
AF = mybir.ActivationFunctionType

B, H, I = 64, 1024, 512
NCORES = 8
OBLK = 2      # o-blocks of 128 per core -> 256 o-rows
BLOC = 32     # samples per core
KCH = 8       # h chunks of 128
MIN_TAU = 1e-3

# Results of the last run_bass_kernel_spmd call (for test harnesses to read
# exec_time_ns when run with BASS_TRACE=1).
LAST_RESULTS = None


def _split_multiwait_instructions(nc):
    """The walrus build here encodes at most one sync-wait per instruction.
    Move extra waits onto single-wait NoOps inserted just before, same
    engine, preserving program order (semantically identical)."""
    for f in nc.m.functions:
        for blk in f.blocks:
            out = []
            changed = False
            for inst in blk.instructions:
                si = inst.sync_info
                if si is not None and si.on_wait and len(si.on_wait) > 1:
                    waits = list(si.on_wait)
                    updates = list(si.on_update or [])
                    for j, w in enumerate(waits[:-1]):
                        nop = mybir.InstNoOp(name=f"{inst.name}-ws{j}", ins=[], outs=[])
                        nop.engine = inst.engine
                        nop.sync_info = bass_rust.SyncInfo(on_wait=[w], on_update=[])
                        out.append(nop)
                    inst.sync_info = bass_rust.SyncInfo(
                        on_wait=[waits[-1]], on_update=updates
                    )
                    changed = True
                out.append(inst)
            if changed:
                blk.instructions = out
    return nc


def _build(repeat=1):
    nc = bass.Bass()
    g_in = nc.dram_tensor("g_sl", [BLOC, OBLK * 128, H], F16, kind="ExternalInput")
    w_in = nc.dram_tensor("w_sl", [OBLK * 128, H], F16, kind="ExternalInput")
    id16_in = nc.dram_tensor("ident16", [128, 128], F16, kind="ExternalInput")
    st_in = nc.dram_tensor("st_sl", [KCH, 128, 2 * BLOC], BF16, kind="ExternalInput")
    xt_in = nc.dram_tensor("xT_sl", [I, BLOC], F16, kind="ExternalInput")
    wih_in = nc.dram_tensor("wihT_sl", [I, OBLK * 128], F16, kind="ExternalInput")
    b_in = nc.dram_tensor("b_sl", [128, OBLK], F32, kind="ExternalInput")
    id32_in = nc.dram_tensor("ident32", [128, 128], F32, kind="ExternalInput")
    temp_in = nc.dram_tensor("temp", [128, 1], F32, kind="ExternalInput")
    h_out = nc.dram_tensor("h_sl", [BLOC, OBLK * 128], F32, kind="ExternalOutput")

    with tile.TileContext(nc) as tc:
        with (
            tc.tile_pool(name="cons", bufs=1) as cons,
            tc.tile_pool(name="lwork", bufs=3) as lwork,
            tc.tile_pool(name="ework", bufs=10) as ework,
            tc.tile_pool(name="tailsb", bufs=1) as tailsb,
            tc.tile_pool(name="ltp_ps", bufs=3, space="PSUM") as ltp_ps,
            tc.tile_pool(name="acc_ps", bufs=2, space="PSUM") as acc_ps,
        ):
            # ---------------- setup ----------------
            # All stationaries ride the Scalar-engine HWDGE queue so the sync
            # queue carries nothing but the gumbel stream (its first dispatch
            # gates the whole pipeline).
            temp128 = cons.tile([128, 1], F32)
            nc.scalar.dma_start(temp128[:], temp_in[:])
            wt_sb = cons.tile([128, OBLK, H], F16)
            nc.scalar.dma_start(
                wt_sb[:], w_in.ap().rearrange("(i p) h -> p i h", p=128)
            )
            # preload the activation table (Exp) early so the 1.3us
            # ACT_TABLE_LOAD doesn't sit in front of the first real exp
            dummy = cons.tile([128, 1], F32)
            nc.scalar.activation(dummy[:], temp128[:], AF.Exp)

            tau128 = cons.tile([128, 1], F32)
            nc.vector.tensor_scalar_max(tau128[:], temp128[:], MIN_TAU)
            rtau128 = cons.tile([128, 1], F32)
            nc.vector.reciprocal(rtau128[:], tau128[:])

            # Wtau = W_hh[o_blk] / tau, in fp16 for the 2x DVE add
            wtau_sb = cons.tile([128, OBLK, H], F16)
            nc.vector.tensor_scalar_mul(wtau_sb[:], wt_sb[:], rtau128[:])

            ident16 = cons.tile([128, 128], F16)
            nc.scalar.dma_start(ident16[:], id16_in[:])
            ident32 = cons.tile([128, 128], F32)
            nc.scalar.dma_start(ident32[:], id32_in[:])
            st_sb = cons.tile([128, KCH, 2 * BLOC], BF16)
            nc.scalar.dma_start(st_sb[:], st_in.ap().rearrange("k p m -> p k m"))
            xt_sb = cons.tile([128, 4, BLOC], F16)
            nc.scalar.dma_start(
                xt_sb[:], xt_in.ap().rearrange("(k p) b -> p k b", p=128)
            )
            wih_sb = cons.tile([128, 4, OBLK * 128], F16)
            nc.scalar.dma_start(
                wih_sb[:], wih_in.ap().rearrange("(k p) o -> p k o", p=128)
            )
            bias_sb = cons.tile([128, OBLK], F32)
            nc.scalar.dma_start(bias_sb[:], b_in[:])

            # ic_T[i] = W_ih[o_blk_i] @ x^T + b  -> (128 o, BLOC b) per block
            # (runs in the PE preamble window before the first logits arrive)
            ic_sb = cons.tile([128, OBLK, BLOC], F32)
            for i in range(OBLK):
                ic_ps = acc_ps.tile([128, OBLK * 128], F32, tag="acc")
                for k in range(4):
                    nc.tensor.matmul(
                        ic_ps[:, :BLOC],
                        wih_sb[:, k, 128 * i : 128 * (i + 1)],
                        xt_sb[:, k, :],
                        start=(k == 0),
                        stop=(k == 3),
                    )
                nc.scalar.activation(
                    ic_sb[:, i, :], ic_ps[:, :BLOC], AF.Identity,
                    bias=bias_sb[:, i : i + 1],
                )

            ndg_all = tailsb.tile([128, BLOC // 4, OBLK * 128], F32)

            def _one_pass():
                contrib = tailsb.tile([128, OBLK, BLOC], F32)

                # ---- main loop: groups of 4 samples (PE column-groups),
                # contraction software-pipelined ONE GROUP BEHIND the
                # transpose+exp front so the PE never stalls on ScalarE ----
                def _contract(grp, ets):
                    acc = acc_ps.tile([128, OBLK * 128], F32, tag="acc")
                    for k in range(KCH):
                        for s in range(4):
                            b = 4 * grp + s
                            nc.tensor.matmul(
                                acc[32 * s : 32 * s + 2, :],
                                st_sb[:, k, 2 * b : 2 * b + 2],
                                ets[s][:, k, :, :],
                                start=(k == 0),
                                stop=(k == KCH - 1),
                                tile_position=(0, 32 * s),
                            )
                    nc.vector.tensor_copy(ndg_all[:, grp, :], acc[:])
                    # per-group tail, overlapped with the next group's work:
                    # transpose num/den back to o-partitions and divide
                    ndT_ps = acc_ps.tile([128, OBLK * 128], F32, tag="acc")
                    for i in range(OBLK):
                        nc.tensor.transpose(
                            ndT_ps[:, 128 * i : 128 * (i + 1)],
                            ndg_all[:, grp, 128 * i : 128 * (i + 1)],
                            ident32[:],
                        )
                    ndT = tailsb.tile([128, OBLK * 128], F32, bufs=2, tag="ndT")
                    nc.vector.tensor_copy(ndT[:], ndT_ps[:])
                    for i in range(OBLK):
                        rec = tailsb.tile([128, 4], F32, bufs=2, tag="rec")
                        nc.vector.reciprocal(
                            rec[:], ndT[:, 128 * i + 1 : 128 * (i + 1) : 32]
                        )
                        nc.vector.tensor_mul(
                            contrib[:, i, 4 * grp : 4 * grp + 4],
                            ndT[:, 128 * i : 128 * (i + 1) : 32],
                            rec[:],
                        )

                pending = None  # (grp, ets) whose contraction is deferred
                for grp in range(BLOC // 4):
                    ets = []
                    for s in range(4):
                        b = 4 * grp + s
                        gt = lwork.tile([128, OBLK, H], F16, bufs=8, tag="gt")
                        nc.sync.dma_start(
                            gt[:], g_in.ap()[b].rearrange("(i p) h -> p i h", p=128)
                        )
                        lt = lwork.tile([128, OBLK, H], F16, tag="lt")
                        nc.vector.tensor_add(lt[:], gt[:], wtau_sb[:])

                        # 16 fp16 transposes fill one 2-bank PSUM tile
                        ltp = ltp_ps.tile([128, KCH, OBLK, 128], F16)
                        for k in range(KCH):
                            for i in range(OBLK):
                                nc.tensor.transpose(
                                    ltp[:, k, i, :],
                                    lt[:, i, 128 * k : 128 * (k + 1)],
                                    ident16[:],
                                )
                        # one 2048-elem exp per sample, PSUM fp16 -> SBUF bf16
                        eth = ework.tile([128, KCH, OBLK, 128], BF16, tag="eth")
                        nc.scalar.activation(eth[:], ltp[:], AF.Exp)
                        ets.append(eth)

                    if pending is not None:
                        _contract(*pending)
                    pending = (grp, ets)
                _contract(*pending)

                # ---- final tail: tanh + output transpose ----
                hout = tailsb.tile([BLOC, OBLK, 128], F32)
                for i in range(OBLK):
                    hpre = tailsb.tile([128, BLOC], F32)
                    nc.vector.tensor_add(hpre[:], contrib[:, i, :], ic_sb[:, i, :])
                    ht = tailsb.tile([128, BLOC], F32)
                    nc.scalar.activation(ht[:], hpre[:], AF.Tanh)
                    hT_ps = acc_ps.tile([128, OBLK * 128], F32, tag="acc")
                    nc.tensor.transpose(
                        hT_ps[:BLOC, :128], ht[:], ident32[:]
                    )
                    nc.vector.tensor_copy(hout[:, i, :], hT_ps[:BLOC, :128])
                    nc.sync.dma_start(
                        h_out.ap()[:, 128 * i : 128 * (i + 1)], hout[:, i, :]
                    )

            for _rep in range(repeat):
                _one_pass()

    _split_multiwait_instructions(nc)
    return nc


def kernel(x_t, h_prev, W_ih, b_ih, W_hh, temperature, gumbel_noise):
    global LAST_RESULTS
    x_t = np.asarray(x_t, dtype=np.float32)
    h_prev = np.asarray(h_prev, dtype=np.float32)
    W_ih = np.asarray(W_ih, dtype=np.float32)
    b_ih = np.asarray(b_ih, dtype=np.float32)
    W_hh = np.asarray(W_hh, dtype=np.float32)
    temperature = np.asarray(temperature, dtype=np.float32)
    gumbel_noise = np.asarray(gumbel_noise, dtype=np.float32)

    nc = _build()

    ident32 = np.eye(128, dtype=np.float32)
    temp_arr = np.full((128, 1), float(temperature), dtype=np.float32)
    g16 = gumbel_noise.astype(np.float16)

    in_maps = []
    for c in range(NCORES):
        q, hb = divmod(c, 2)
        o0 = OBLK * 128 * q
        b0 = BLOC * hb
        g_sl = np.ascontiguousarray(g16[b0 : b0 + BLOC, o0 : o0 + OBLK * 128, :])
        w_sl = np.ascontiguousarray(W_hh[o0 : o0 + OBLK * 128, :]).astype(np.float16)
        st_sl = np.ones((KCH, 128, 2 * BLOC), np.float32)
        st_sl[:, :, 0::2] = np.ascontiguousarray(h_prev[b0 : b0 + BLOC].T).reshape(
            KCH, 128, BLOC
        )
        xT_sl = np.ascontiguousarray(x_t[b0 : b0 + BLOC].T).astype(np.float16)
        wihT_sl = np.ascontiguousarray(W_ih[o0 : o0 + OBLK * 128].T).astype(np.float16)
        b_sl = np.ascontiguousarray(b_ih[o0 : o0 + OBLK * 128].reshape(OBLK, 128).T)
        in_maps.append(
            {
                "g_sl": g_sl,
                "w_sl": w_sl,
                "ident16": np.eye(128, dtype=np.float16),
                "st_sl": st_sl.astype(ml_dtypes.bfloat16),
                "xT_sl": xT_sl,
                "wihT_sl": wihT_sl,
                "b_sl": b_sl,
                "ident32": ident32,
                "temp": temp_arr,
            }
        )

    res = run_bass_kernel_spmd(nc, in_maps, list(range(NCORES)))
    LAST_RESULTS = res

    h = np.empty((B, H), np.float32)
    for c in range(NCORES):
        q, hb = divmod(c, 2)
        o0 = OBLK * 128 * q
        b0 = BLOC * hb
        h[b0 : b0 + BLOC, o0 : o0 + OBLK * 128] = res.results[c]["h_sl"]
    return h
